# revision 12
# baseline (speedup 1.0000x reference)
"""Trainium2 Bass kernel for nn_LoadPathLoss.

reference computation:
  structure = state[:, ch]                  # [B=4, D=64, H=128, W=128]
  s = structure[:, 0]
  for z in 1..63:  s = max(s, min(structure[:, z], maxpool3x3(s)))
  return relu(structure - s[:, None]).mean()

Strategy: pure data parallel over B=4 on 4 NeuronCores, one batch element per
core.  The z-scan is a 63-step serial chain of [128, 128] ops per core:
  - H-direction max3 via two PE shift-matmuls (shifted-identity lhsT) into
    PSUM + two DVE maxes.  Values are kept offset by +16 so the matmul's
    zero boundary rows act as -inf.
  - W-direction max3 via two DVE maxes on a zero-padded [128, 130] tile.
  - step update uses max(S, min(c', below)) == min(below, max(S, c'))
    (valid since below = maxpool3x3(S) >= S), with g = max(S, c'+16) as one
    scalar_tensor_tensor off the pool chain.
The final mean uses relu(c - s) = max(c, s) - s: one scalar_tensor_tensor per
z-chunk with accum_out produces per-partition sums of max(c, s); host combines
partials in f64.
"""

import numpy as np

B, C, D, H, W = 4, 8, 64, 128, 128
NCORES = 4
ZCHUNK = 8          # z-slices per DMA chunk / phase-2 op
NCHUNK = D // ZCHUNK
SHIFT = 16.0

_cached = {}


def _build_nc(d_steps=D, phase2=True, use_pe=True, use_gps=False):
    import concourse.bacc as bacc
    import concourse.mybir as mybir
    from concourse.tile import TileContext

    fp32 = mybir.dt.float32
    mx = mybir.AluOpType.max

    nc = bacc.Bacc("TRN2", target_bir_lowering=False, debug=False)
    cb = nc.dram_tensor("cb", [D, H, W], fp32, kind="ExternalInput")
    shifts = nc.dram_tensor("shifts", [H, 2 * H], fp32, kind="ExternalInput")
    out = nc.dram_tensor("out", [H, NCHUNK + 1], fp32, kind="ExternalOutput")

    with TileContext(nc) as tc:
        with (
            tc.tile_pool(name="sbuf", bufs=1) as pool,
            tc.tile_pool(name="psum", bufs=2, space="PSUM") as psum,
        ):
            sh0 = pool.tile([H, 2 * H], fp32, tag="sh0")
            sh = pool.tile([H, 2 * H], fp32, tag="sh")
            chunks = [
                pool.tile([H, ZCHUNK, W], fp32, tag=f"cb{k}", name=f"cb{k}") for k in range(NCHUNK)
            ]
            S = pool.tile([H, W], fp32, tag="S")
            hp = pool.tile([H, W + 2], fp32, tag="hp")
            t129 = pool.tile([H, W + 1], fp32, tag="t129")
            below = pool.tile([H, W], fp32, tag="below")
            m = pool.tile([H, W], fp32, tag="m")
            sraw = pool.tile([H, W], fp32, tag="sraw")
            acc = pool.tile([H, NCHUNK + 1], fp32, tag="acc")

            # weights: DMA then DVE-copy shield so matmuls wait on DVE only
            nc.sync.dma_start(out=sh0[:], in_=shifts[:, :])
            nc.vector.tensor_copy(sh[:], sh0[:])

            # input chunks: cb[z,h,w] -> sbuf [h, z, w]
            for k in range(NCHUNK):
                src = cb[k * ZCHUNK : (k + 1) * ZCHUNK].rearrange("z h w -> h z w")
                nc.sync.dma_start(out=chunks[k][:], in_=src)

            # zero-pad borders of hp once; center overwritten every step
            nc.vector.memset(hp[:], 0.0)

            # S = c_0 + 16
            nc.vector.tensor_scalar_add(S[:], chunks[0][:, 0, :], SHIFT)

            for z in range(1, d_steps):
                k, j = z // ZCHUNK, z % ZCHUNK
                c_z = chunks[k][:, j, :]
                ps = psum.tile([H, 2 * W], fp32, tag="ps", name=f"ps{z}")
                f32r = mybir.dt.float32r
                if use_pe:
                    nc.tensor.matmul(
                        out=ps[:, 0:W], lhsT=sh[:, 0:H], rhs=S[:],
                        start=True, stop=True,
                    )
                    nc.tensor.matmul(
                        out=ps[:, W : 2 * W], lhsT=sh[:, H : 2 * H], rhs=S[:],
                        start=True, stop=True,
                    )
                    up_ap, dn_ap = ps[:, 0:W], ps[:, W : 2 * W]
                else:
                    up_ap, dn_ap = sh[:, 0:W], sh[:, W : 2 * W]
                # g = max(S, c_z + 16) on GPSIMD, parallel to the DVE pool chain
                # (uses max(S, min(c', below)) == min(below, max(S, c')),
                #  valid because below = maxpool3x3(S) >= S)
                geng = nc.gpsimd if use_gps else nc.vector
                geng.scalar_tensor_tensor(
                    out=m[:], in0=c_z, scalar=SHIFT, in1=S[:],
                    op0=mybir.AluOpType.add, op1=mx,
                )
                nc.vector.tensor_tensor(
                    out=hp[:, 1 : W + 1], in0=S[:], in1=up_ap, op=mx
                )
                nc.vector.tensor_tensor(
                    out=hp[:, 1 : W + 1], in0=hp[:, 1 : W + 1],
                    in1=dn_ap, op=mx,
                )
                nc.vector.tensor_tensor(
                    out=t129[:], in0=hp[:, 0 : W + 1], in1=hp[:, 1 : W + 2], op=mx
                )
                nc.vector.tensor_tensor(
                    out=below[:], in0=t129[:, 0:W], in1=hp[:, 2 : W + 2], op=mx
                )
                nc.vector.tensor_tensor(
                    out=S[:], in0=below[:], in1=m[:], op=mybir.AluOpType.min
                )

            # phase 2: sum over z,w of max(c, s); and sum of s
            nc.vector.tensor_scalar_add(sraw[:], S[:], -SHIFT)
            nc.vector.tensor_reduce(
                out=acc[:, NCHUNK : NCHUNK + 1], in_=sraw[:],
                axis=mybir.AxisListType.X, op=mybir.AluOpType.add,
            )
            sbc = sraw[:].unsqueeze(1).broadcast_to((H, ZCHUNK, W))
            for k in range(NCHUNK if phase2 else 0):
                nc.vector.scalar_tensor_tensor(
                    out=chunks[k][:], in0=chunks[k][:], scalar=0.0, in1=sbc,
                    op0=mybir.AluOpType.bypass, op1=mx,
                    accum_out=acc[:, k : k + 1],
                )

            nc.sync.dma_start(out=out[:, :], in_=acc[:])

    nc.compile()
    return nc


def _shift_mats():
    U = np.zeros((H, H), dtype=np.float32)  # lhsT: out[p] = x[p+1]
    Dm = np.zeros((H, H), dtype=np.float32)  # lhsT: out[p] = x[p-1]
    for p in range(H - 1):
        U[p + 1, p] = 1.0
        Dm[p, p + 1] = 1.0
    return np.concatenate([U, Dm], axis=1)


def _make_runner(nc):
    """Cached multi-core PJRT runner (mirrors bass2jax.run_bass_via_pjrt but
    keeps the jitted shard_map so repeat calls skip retrace/recompile)."""
    import jax
    from jax.sharding import Mesh, PartitionSpec
    from jax.experimental.shard_map import shard_map
    import concourse.mybir as mybir
    from concourse import bass2jax

    bass2jax.install_neuronx_cc_hook()

    partition_name = nc.partition_id_tensor.name if nc.partition_id_tensor else None
    in_names, out_names, out_avals, zero_outs = [], [], [], []
    for alloc in nc.m.functions[0].allocations:
        if not isinstance(alloc, mybir.MemoryLocationSet):
            continue
        name = alloc.memorylocations[0].name
        if alloc.kind == "ExternalInput":
            if name != partition_name:
                in_names.append(name)
        elif alloc.kind == "ExternalOutput":
            shape = tuple(alloc.tensor_shape)
            dtype = mybir.dt.np(alloc.dtype)
            out_names.append(name)
            out_avals.append(jax.core.ShapedArray(shape, dtype))
            zero_outs.append(np.zeros(shape, dtype))
    n_params = len(in_names)
    n_outs = len(out_avals)
    all_names = in_names + out_names
    donate = tuple(range(n_params, n_params + n_outs))

    def _body(*args):
        operands = list(args)
        if partition_name is not None:
            operands.append(bass2jax.partition_id_tensor())
        outs = bass2jax._bass_exec_p.bind(
            *operands,
            out_avals=tuple(out_avals),
            in_names=tuple(all_names + ([partition_name] if partition_name else [])),
            out_names=tuple(out_names),
            lowering_input_output_aliases=(),
            sim_require_finite=True,
            sim_require_nnan=True,
            nc=nc,
        )
        return tuple(outs)

    devices = jax.devices()[:NCORES]
    mesh = Mesh(np.asarray(devices), ("core",))
    in_specs = (PartitionSpec("core"),) * (n_params + n_outs)
    out_specs = (PartitionSpec("core"),) * n_outs
    sharded = jax.jit(
        shard_map(_body, mesh=mesh, in_specs=in_specs, out_specs=out_specs,
                  check_rep=False),
        donate_argnums=donate, keep_unused=True,
    )

    def run(in_maps):
        args = [
            np.concatenate([np.asarray(m[name]) for m in in_maps], axis=0)
            for name in in_names
        ]
        zouts = [np.concatenate([z] * NCORES, axis=0) for z in zero_outs]
        outs = sharded(*args, *zouts)
        res = []
        for b in range(NCORES):
            d = {}
            for i, name in enumerate(out_names):
                full = np.asarray(outs[i])
                per = full.shape[0] // NCORES
                d[name] = full[b * per : (b + 1) * per]
            res.append(d)
        return res

    return run


def kernel(state, ch_structure):
    if "nc" not in _cached:
        _cached["nc"] = _build_nc()
        _cached["run"] = _make_runner(_cached["nc"])

    structure = np.ascontiguousarray(state[:, int(ch_structure)], dtype=np.float32)
    sh = _shift_mats()
    in_maps = [{"cb": structure[b], "shifts": sh} for b in range(NCORES)]
    results = _cached["run"](in_maps)
    _cached["last"] = results

    total = 0.0
    for b in range(NCORES):
        o = results[b]["out"].astype(np.float64)
        total += o[:, :NCHUNK].sum() - float(D) * o[:, NCHUNK].sum()
    mean = total / float(B * D * H * W)
    return np.asarray(mean, dtype=np.float32)


if __name__ == "__main__":
    rng = np.random.default_rng(0)
    st = rng.standard_normal((B, C, D, H, W)).astype(np.float32)
    print(kernel(st, 3))
